# revision 1
# baseline (speedup 1.0000x reference)
"""Multi-head attention (dense_transformer) on 8 Trainium2 NeuronCores.

Reference computation (DIM=1024, HEADS=16, HEAD_DIM=64, SCALE=DIM**-0.5):
    qkv = x @ w_qkv                      # [b, n, 3*dim]
    q, k, v = split-heads(qkv)           # each [b, h, n, d]
    attn = softmax(q @ k^T * SCALE)
    out = (attn @ v) re-merged @ w_out + b_out

Sharding: 8 cores = (batch b in 0..3) x (head-group hg in 0..1, 8 heads each).
Each core computes a [2048, 1024] fp32 partial of the output projection for
its (batch, head-group); host sums the two head-group partials and adds bias.

Per-core dataflow (all matmuls fp16 operands, fp32 PSUM accumulate):
    qkT = wqk^T @ x^T      [1024, 2048]  (Q^T rows 0-511, K^T rows 512-1023)
    V   = x @ wv           [2048, 512]
    per (i-block of 512 queries, head pair):
        S^T[j, i] = K_h Q_h^T            (row-tiled pair: K=64 each)
        E = exp(S^T / 32)                (ScalarE, straight from PSUM)
        U^T[d, i] += V_h^T E             (col-tiled pair: M=64+64)
        r[i] = sum_j E[j, i]             (DVE chunk adds + ones-matmul reduce)
        U^T *= 1/r broadcast             (K=1 ones matmul broadcast)
    Y = U^T.T @ wout partial -> DRAM fp32
"""

import numpy as np

P = 128
DIM = 1024
NT = 2048          # tokens per batch
HL = 8             # heads per core (local)
HD = 64
KD = DIM // P      # 8 contraction chunks for the projections
NI = NT // 512     # 4 query blocks of 512
NJ = NT // P       # 16 key chunks of 128
SCALE = DIM ** -0.5

_CACHE = {}

def _build(loop_iters=None):
    from contextlib import ExitStack

    import concourse.bacc as bacc
    import concourse.tile as tile
    from concourse import mybir

    f16 = mybir.dt.float16
    f32 = mybir.dt.float32
    EXP = mybir.ActivationFunctionType.Exp

    nc = bacc.Bacc("TRN2", target_bir_lowering=False, debug=False)

    xT = nc.dram_tensor("xT", [DIM, NT], f32, kind="ExternalInput").ap()
    wqk = nc.dram_tensor("wqk", [DIM, 1024], f32, kind="ExternalInput").ap()
    wv = nc.dram_tensor("wv", [DIM, 512], f32, kind="ExternalInput").ap()
    wout = nc.dram_tensor("wout", [512, 1024], f32, kind="ExternalInput").ap()
    y = nc.dram_tensor("y", [NT, 1024], f32, kind="ExternalOutput").ap()

    with tile.TileContext(nc) as tc, ExitStack() as ctx, nc.allow_low_precision(
        reason="fp16 softmax-denominator accumulation, validated vs reference"
    ):
        persist = ctx.enter_context(tc.tile_pool(name="persist", bufs=1))
        stage = ctx.enter_context(tc.tile_pool(name="stage", bufs=2))
        epool = ctx.enter_context(tc.tile_pool(name="epool", bufs=8))
        rpool = ctx.enter_context(tc.tile_pool(name="rpool", bufs=4))
        ypool = ctx.enter_context(tc.tile_pool(name="ypool", bufs=3))
        ps_s = ctx.enter_context(tc.tile_pool(name="ps_s", bufs=2, space="PSUM"))
        ps_u = ctx.enter_context(tc.tile_pool(name="ps_u", bufs=2, space="PSUM"))
        ps_r = ctx.enter_context(tc.tile_pool(name="ps_r", bufs=2, space="PSUM"))

        xT_t = persist.tile([P, KD, NT], f16)        # x^T, fp16
        qkT_t = persist.tile([P, KD, NT], f16)       # [Q^T; K^T]
        V_t = persist.tile([P, NJ, 512], f16)        # V natural layout
        U_t = persist.tile([P, 4, NT], f16)          # U^T normalized, pair-chunked
        wqk_t = persist.tile([P, KD, 1024], f16)
        wv_t = persist.tile([P, KD, 512], f16)
        wout_t = persist.tile([P, 4, 1024], f16)
        ones_r = persist.tile([P, 1], f16)           # K=128, M=1 column-sum
        ones_b = persist.tile([1, 64], f16)          # K=1 broadcast
        nc.vector.memset(ones_r, 1.0)
        nc.vector.memset(ones_b, 1.0)

        def load_cast(dst, src_ap, cols):
            st = stage.tile([P, 2048], f32, tag="stage", name="st")
            nc.sync.dma_start(out=st[:, :cols], in_=src_ap)
            nc.gpsimd.tensor_copy(out=dst, in_=st[:, :cols])

        def qk_proj_chunk(m, n, pool, tag):
            ps = pool.tile([P, 512], f32, tag=tag, name="ps_qk")
            for k in range(KD):
                nc.tensor.matmul(
                    ps,
                    lhsT=wqk_t[:, k, m * P:(m + 1) * P],
                    rhs=xT_t[:, k, n * 512:(n + 1) * 512],
                    start=(k == 0), stop=(k == KD - 1),
                )
            nc.vector.tensor_copy(
                out=qkT_t[:, m, n * 512:(n + 1) * 512], in_=ps
            )

        def v_proj(mt, pool, tag):
            ps = pool.tile([P, 512], f32, tag=tag, name="ps_v")
            for k in range(KD):
                nc.tensor.matmul(
                    ps,
                    lhsT=xT_t[:, k, mt * P:(mt + 1) * P],
                    rhs=wv_t[:, k, :],
                    start=(k == 0), stop=(k == KD - 1),
                )
            nc.vector.tensor_copy(out=V_t[:, mt, :], in_=ps)

        def body(_iv=None):
            # ---- weight + x loads (fp32 DMA, cast to fp16) ----
            for k in range(KD):
                load_cast(wqk_t[:, k, :], wqk[k * P:(k + 1) * P, :], 1024)
            for k in range(KD):
                load_cast(wv_t[:, k, :], wv[k * P:(k + 1) * P, :], 512)
            for k in range(4):
                load_cast(wout_t[:, k, :], wout[k * P:(k + 1) * P, :], 1024)
            for k in range(KD):
                load_cast(xT_t[:, k, :], xT[k * P:(k + 1) * P, :], NT)

            # ---- projections needed before the pipeline starts ----
            for n in range(NI):
                qk_proj_chunk(0, n, ps_s, "s")
            for n in range(NI):
                qk_proj_chunk(4, n, ps_s, "s")
            for mt in range(8):
                v_proj(mt, ps_s, "s")

            # Remaining QKV work, interleaved into block-0 pipeline groups.
            # Pair p's qkT chunks must be fully emitted before its lookahead
            # S^T (2 groups before the pair starts); V chunk jc before the
            # PV at group jc of pair 0.
            extra_at = {}

            def add_extra(G, fn):
                extra_at.setdefault(G, []).append(fn)

            for jc in range(8, NJ):
                add_extra(jc - 8, lambda mt=jc: v_proj(mt, ps_r, "rr"))
            for pi, (ma, mb) in enumerate([(1, 5), (2, 6), (3, 7)]):
                g0 = pi * 16 + 2
                seq = [(ma, 0), (mb, 0), (ma, 1), (mb, 1),
                       (ma, 2), (mb, 2), (ma, 3), (mb, 3)]
                for t, (m, n) in enumerate(seq):
                    add_extra(
                        g0 + t,
                        lambda m=m, n=n: qk_proj_chunk(m, n, ps_r, "rr"),
                    )

            # ---- attention: flat software pipeline over (i, p, g) ----
            order = [
                (i, p, g) for i in range(NI) for p in range(4) for g in range(NJ)
            ]

            def st_group(i, p, g):
                """S^T for one j-chunk, both heads of the pair, row-tiled."""
                isl = slice(i * 512, (i + 1) * 512)
                jsl = slice(g * P, (g + 1) * P)
                s = ps_s.tile([P, 2, 512], f32, tag="s", name="s_ps")
                for hh in range(2):
                    pb = hh * 64
                    nc.tensor.matmul(
                        s[:, hh, :],
                        lhsT=qkT_t[pb:pb + 64, 4 + p, jsl],
                        rhs=qkT_t[pb:pb + 64, p, isl],
                        start=True, stop=True,
                        tile_position=(pb, 0),
                    )
                return s

            def pair_tail(i, p, u, racc):
                isl = slice(i * 512, (i + 1) * 512)
                rb = ps_r.tile([P, 512], f32, tag="rr", name="rb")
                for hh in range(2):
                    rp = ps_r.tile([P, 512], f32, tag="rr", name="rp")
                    nc.tensor.matmul(
                        rp[0:1, :], lhsT=ones_r, rhs=racc[:, hh, :],
                        start=True, stop=True,
                    )
                    rs = rpool.tile([1, 512], f16, tag=f"rs{hh}", name="rs")
                    nc.vector.reciprocal(out=rs, in_=rp[0:1, :])
                    nc.tensor.matmul(
                        rb[hh * 64:(hh + 1) * 64, :],
                        lhsT=ones_b, rhs=rs,
                        start=True, stop=True,
                        tile_position=(0, hh * 64),
                    )
                rb_sb = rpool.tile([P, 512], f16, tag="rb", name="rb_sb")
                nc.vector.tensor_copy(out=rb_sb, in_=rb)
                nc.vector.tensor_mul(out=U_t[:, p, isl], in0=u, in1=rb_sb)

            def out_proj_chunk(i, m, n2):
                msl = slice(i * 512 + m * P, i * 512 + (m + 1) * P)
                py = ps_r.tile([P, 512], f32, tag="rr", name="py")
                for k in range(4):
                    nc.tensor.matmul(
                        py,
                        lhsT=U_t[:, k, msl],
                        rhs=wout_t[:, k, n2 * 512:(n2 + 1) * 512],
                        start=(k == 0), stop=(k == 3),
                    )
                ysb = ypool.tile([P, 512], f32, tag="y", name="ysb")
                nc.vector.tensor_copy(out=ysb, in_=py)
                nc.sync.dma_start(
                    out=y[msl, n2 * 512:(n2 + 1) * 512], in_=ysb
                )

            late_at = {}
            for i in range(NI - 1):
                for t, (m, n2) in enumerate(
                    [(m, n2) for m in range(4) for n2 in range(2)]
                ):
                    # block i's out-proj runs during block i+1, one chunk
                    # every other group
                    late_at.setdefault((i + 1) * 64 + 2 * t + 1, []).append(
                        lambda i=i, m=m, n2=n2: out_proj_chunk(i, m, n2)
                    )

            s_tiles = {0: st_group(*order[0]), 1: st_group(*order[1])}
            e_tiles = {}
            u = None
            racc = None
            # One-step software pipeline: group G's exp is issued at step G,
            # its PV + denominator add at step G+1 (after the lookahead S^T),
            # so PE never queues a matmul behind a just-issued exp.
            for G in range(len(order) + 1):
                if G < len(order):
                    s = s_tiles.pop(G)
                    e = epool.tile([P, 2, 512], f16, tag="e", name="e")
                    nc.scalar.activation(
                        out=e[:], in_=s[:], func=EXP, scale=SCALE
                    )
                    e_tiles[G] = e
                    if G + 2 < len(order):
                        s_tiles[G + 2] = st_group(*order[G + 2])
                    for fn in extra_at.get(G, ()):
                        fn()
                if G >= 1:
                    i0, p0, g0 = order[G - 1]
                    e0 = e_tiles.pop(G - 1)
                    if g0 == 0:
                        u = ps_u.tile([P, 512], f32, tag="u", name="u")
                        racc = rpool.tile([P, 2, 512], f16, tag="r", name="racc")
                    for hh in range(2):
                        h = 2 * p0 + hh
                        nc.tensor.matmul(
                            u[hh * 64:(hh + 1) * 64, :],
                            lhsT=V_t[:, g0, h * 64:(h + 1) * 64],
                            rhs=e0[:, hh, :],
                            start=(g0 == 0), stop=(g0 == NJ - 1),
                            tile_position=(0, hh * 64),
                            skip_group_check=True,
                        )
                    if g0 == 0:
                        nc.vector.tensor_copy(out=racc[:], in_=e0[:])
                    else:
                        nc.vector.tensor_add(racc[:], racc[:], e0[:])
                    if g0 == NJ - 1:
                        pair_tail(i0, p0, u, racc)
                for fn in late_at.get(G, ()):
                    fn()
            for m in range(4):
                for n2 in range(2):
                    out_proj_chunk(NI - 1, m, n2)

        if loop_iters is None:
            body()
        else:
            with tc.For_i(0, loop_iters, 1) as iv:
                body(iv)

    nc.compile()
    return nc


def _in_maps(x, w_qkv, w_out):
    in_maps = []
    for bi in range(4):
        xTb = np.ascontiguousarray(x[bi].T)
        for hg in range(2):
            c = slice(hg * 512, (hg + 1) * 512)
            wqk = np.ascontiguousarray(
                np.concatenate([w_qkv[:, c], w_qkv[:, 1024:2048][:, c]], axis=1)
            )
            wv = np.ascontiguousarray(w_qkv[:, 2048:3072][:, c])
            wo = np.ascontiguousarray(w_out[c, :])
            in_maps.append({"xT": xTb, "wqk": wqk, "wv": wv, "wout": wo})
    return in_maps


def kernel(x, w_qkv, w_out, b_out):
    from concourse.bass_utils import run_bass_kernel_spmd

    if "nc" not in _CACHE:
        _CACHE["nc"] = _build()
    nc = _CACHE["nc"]

    x = np.ascontiguousarray(np.asarray(x, dtype=np.float32))
    w_qkv = np.asarray(w_qkv, dtype=np.float32)
    w_out = np.asarray(w_out, dtype=np.float32)
    b_out = np.asarray(b_out, dtype=np.float32)

    res = run_bass_kernel_spmd(
        nc, _in_maps(x, w_qkv, w_out), core_ids=list(range(8))
    )
    out = np.empty((4, NT, DIM), dtype=np.float32)
    for bi in range(4):
        out[bi] = res.results[2 * bi]["y"] + res.results[2 * bi + 1]["y"] + b_out
    return out



# revision 2
# speedup vs baseline: 1.3662x; 1.3662x over previous
"""Multi-head attention (dense_transformer) on 8 Trainium2 NeuronCores.

Reference computation (DIM=1024, HEADS=16, HEAD_DIM=64, SCALE=DIM**-0.5):
    qkv = x @ w_qkv                      # [b, n, 3*dim]
    q, k, v = split-heads(qkv)           # each [b, h, n, d]
    attn = softmax(q @ k^T * SCALE)
    out = (attn @ v) re-merged @ w_out + b_out

Sharding: 8 cores = (batch b in 0..3) x (head-group hg in 0..1, 8 heads each).
Each core computes a [2048, 1024] f16 partial of the output projection for
its (batch, head-group); host sums the two head-group partials and adds bias.

v2 design notes (vs v1):
  - all inputs cast to f16 on host: halves load DMA, kills gpsimd casts
  - loads issued per-chunk in compute-dependency order so the first
    projection matmuls start ~1us in; ScalarE (exp, the pacing engine at
    ~267us busy) starts ~6us in instead of ~92us
  - projection chunks deadline-scheduled into the pipeline groups
  - softmax pair-tail (denominator reduce/recip/broadcast/normalize) is
    deferred 1-2 groups so it never head-blocks the PE queue ahead of the
    next S^T
  - denominator accumulation split: head0 on DVE, head1 on GpSimd
  - y output f16 (host sums partials in fp32)
"""

import numpy as np

P = 128
DIM = 1024
NT = 2048          # tokens per batch
HL = 8             # heads per core (local)
HD = 64
KD = DIM // P      # 8 contraction chunks for the projections
NI = NT // 512     # 4 query blocks of 512
NJ = NT // P       # 16 key chunks of 128
SCALE = DIM ** -0.5

_CACHE = {}


def _build(loop_iters=None, racc_split=True):
    from contextlib import ExitStack

    import concourse.bacc as bacc
    import concourse.tile as tile
    from concourse import mybir

    f16 = mybir.dt.float16
    f32 = mybir.dt.float32
    EXP = mybir.ActivationFunctionType.Exp

    nc = bacc.Bacc("TRN2", target_bir_lowering=False, debug=False)

    xT = nc.dram_tensor("xT", [DIM, NT], f16, kind="ExternalInput").ap()
    wqk = nc.dram_tensor("wqk", [DIM, 1024], f16, kind="ExternalInput").ap()
    wv = nc.dram_tensor("wv", [DIM, 512], f16, kind="ExternalInput").ap()
    wout = nc.dram_tensor("wout", [512, 1024], f16, kind="ExternalInput").ap()
    y = nc.dram_tensor("y", [NT, 1024], f16, kind="ExternalOutput").ap()

    with tile.TileContext(nc) as tc, ExitStack() as ctx, nc.allow_low_precision(
        reason="f16 weights/activations + f16 partial outputs, validated vs reference"
    ):
        persist = ctx.enter_context(tc.tile_pool(name="persist", bufs=1))
        epool = ctx.enter_context(tc.tile_pool(name="epool", bufs=6))
        rpool = ctx.enter_context(tc.tile_pool(name="rpool", bufs=3))
        ypool = ctx.enter_context(tc.tile_pool(name="ypool", bufs=3))
        ps_s = ctx.enter_context(tc.tile_pool(name="ps_s", bufs=2, space="PSUM"))
        ps_u = ctx.enter_context(tc.tile_pool(name="ps_u", bufs=2, space="PSUM"))
        ps_r = ctx.enter_context(tc.tile_pool(name="ps_r", bufs=2, space="PSUM"))

        xT_t = persist.tile([P, KD, NT], f16)        # x^T
        qkT_t = persist.tile([P, KD, NT], f16)       # [Q^T; K^T]
        V_t = persist.tile([P, NJ, 512], f16)        # V natural layout
        U_t = persist.tile([P, 4, NT], f16)          # U^T normalized, pair-chunked
        wqk_t = persist.tile([P, KD, 1024], f16)
        wv_t = persist.tile([P, KD, 512], f16)
        wout_t = persist.tile([P, 4, 1024], f16)
        ones_r = persist.tile([P, 1], f16)           # K=128, M=1 column-sum
        ones_b = persist.tile([1, 64], f16)          # K=1 broadcast
        nc.vector.memset(ones_r, 1.0)
        nc.vector.memset(ones_b, 1.0)

        def body(_iv=None):
            # ---- loads: f16 DMA straight into persist tiles, SP ring,
            # ordered so the first qk projection can start immediately ----
            for k in range(KD):
                nc.sync.dma_start(
                    out=xT_t[:, k, 0:512], in_=xT[k * P:(k + 1) * P, 0:512]
                )
                nc.sync.dma_start(out=wqk_t[:, k, :], in_=wqk[k * P:(k + 1) * P, :])
            nc.sync.dma_start(
                out=wv_t[:, :, :], in_=wv.rearrange("(k p) c -> p k c", p=P)
            )
            for q in range(1, 4):
                for k in range(KD):
                    nc.sync.dma_start(
                        out=xT_t[:, k, q * 512:(q + 1) * 512],
                        in_=xT[k * P:(k + 1) * P, q * 512:(q + 1) * 512],
                    )
            nc.sync.dma_start(
                out=wout_t[:, :, :], in_=wout.rearrange("(k p) c -> p k c", p=P)
            )

            def qk_chunk(m, n):
                ps = ps_r.tile([P, 512], f32, tag="rr", name="ps_qk")
                for k in range(KD):
                    nc.tensor.matmul(
                        ps,
                        lhsT=wqk_t[:, k, m * P:(m + 1) * P],
                        rhs=xT_t[:, k, n * 512:(n + 1) * 512],
                        start=(k == 0), stop=(k == KD - 1),
                    )
                nc.vector.tensor_copy(out=qkT_t[:, m, n * 512:(n + 1) * 512], in_=ps)

            def v_chunk(mt):
                ps = ps_r.tile([P, 512], f32, tag="rr", name="ps_v")
                for k in range(KD):
                    nc.tensor.matmul(
                        ps,
                        lhsT=xT_t[:, k, mt * P:(mt + 1) * P],
                        rhs=wv_t[:, k, :],
                        start=(k == 0), stop=(k == KD - 1),
                    )
                nc.vector.tensor_copy(out=V_t[:, mt, :], in_=ps)

            order = [
                (i, p, g) for i in range(NI) for p in range(4) for g in range(NJ)
            ]
            NG = len(order)

            def st_group(i, p, g):
                """S^T for one j-chunk, both heads of the pair, row-tiled."""
                isl = slice(i * 512, (i + 1) * 512)
                jsl = slice(g * P, (g + 1) * P)
                s = ps_s.tile([P, 2, 512], f32, tag="s", name="s_ps")
                for hh in range(2):
                    pb = hh * 64
                    nc.tensor.matmul(
                        s[:, hh, :],
                        lhsT=qkT_t[pb:pb + 64, 4 + p, jsl],
                        rhs=qkT_t[pb:pb + 64, p, isl],
                        start=True, stop=True,
                        tile_position=(pb, 0),
                    )
                return s

            sched = {}

            def at(G, fn):
                sched.setdefault(G, []).append(fn)

            # Projection tasks with need-by-group deadlines (perf heuristic
            # only; semaphores enforce correctness). Q chunk (p, i) feeds
            # S^T of pair p in block i; K chunk (4+p, n) feeds its j range;
            # V chunk mt feeds PV at group mt.
            tasks = []
            for p in range(4):
                for n in range(4):
                    if (p, n) != (0, 0):
                        tasks.append(
                            (max(0, n * 64 + p * 16 - 6),
                             lambda m=p, nn=n: qk_chunk(m, nn))
                        )
                    if (p, n) != (0, 0):
                        tasks.append(
                            (max(0, p * 16 + 4 * n - 6),
                             lambda m=4 + p, nn=n: qk_chunk(m, nn))
                        )
                    else:
                        pass
            for mt in range(2, NJ):
                tasks.append((max(0, mt - 2), lambda m=mt: v_chunk(m)))
            tasks.sort(key=lambda t: t[0])
            for dl, fn in tasks:
                at(dl, fn)

            # out-projection of block i runs during block i+1
            def out_proj(i, m, n2):
                msl = slice(i * 512 + m * P, i * 512 + (m + 1) * P)
                py = ps_r.tile([P, 512], f32, tag="rr", name="py")
                for k in range(4):
                    nc.tensor.matmul(
                        py,
                        lhsT=U_t[:, k, msl],
                        rhs=wout_t[:, k, n2 * 512:(n2 + 1) * 512],
                        start=(k == 0), stop=(k == 3),
                    )
                ysb = ypool.tile([P, 512], f16, tag="y", name="ysb")
                nc.vector.tensor_copy(out=ysb, in_=py)
                nc.sync.dma_start(out=y[msl, n2 * 512:(n2 + 1) * 512], in_=ysb)

            for i in range(NI - 1):
                for t, (m, n2) in enumerate(
                    [(m, n2) for m in range(4) for n2 in range(2)]
                ):
                    at((i + 1) * 64 + 2 * t + 3,
                       lambda i=i, m=m, n2=n2: out_proj(i, m, n2))

            def tail_rp(racc, rs_pair):
                for hh in range(2):
                    rp = ps_r.tile([P, 512], f32, tag="rr", name="rp")
                    nc.tensor.matmul(
                        rp[0:1, :], lhsT=ones_r, rhs=racc[:, hh, :],
                        start=True, stop=True,
                    )
                    rs = rpool.tile([1, 512], f16, tag=f"rs{hh}", name="rs")
                    nc.vector.reciprocal(out=rs, in_=rp[0:1, :])
                    rs_pair.append(rs)

            def tail_fin(i0, p0, u, rs_pair):
                isl = slice(i0 * 512, (i0 + 1) * 512)
                rb = ps_r.tile([P, 512], f32, tag="rr", name="rb")
                for hh in range(2):
                    nc.tensor.matmul(
                        rb[hh * 64:(hh + 1) * 64, :],
                        lhsT=ones_b, rhs=rs_pair[hh],
                        start=True, stop=True,
                        tile_position=(0, hh * 64),
                    )
                rb_sb = rpool.tile([P, 512], f16, tag="rb", name="rb_sb")
                nc.vector.tensor_copy(out=rb_sb, in_=rb)
                nc.vector.tensor_mul(out=U_t[:, p0, isl], in0=u, in1=rb_sb)

            # ---- preamble: minimum projections for the pipeline start ----
            qk_chunk(0, 0)
            qk_chunk(4, 0)
            v_chunk(0)
            v_chunk(1)

            s_tiles = {0: st_group(*order[0]), 1: st_group(*order[1])}
            e_tiles = {}
            u = None
            racc = None
            # One-step software pipeline: group G's exp is issued at step G,
            # its PV + denominator add at step G+1 (after the lookahead S^T).
            for G in range(NG + 3):
                if 1 <= G <= NG:
                    i0, p0, g0 = order[G - 1]
                    e0 = e_tiles.pop(G - 1)
                    if g0 == 0:
                        u = ps_u.tile([P, 512], f32, tag="u", name="u")
                        racc = rpool.tile([P, 2, 512], f16, tag="r", name="racc")
                    for hh in range(2):
                        h = 2 * p0 + hh
                        nc.tensor.matmul(
                            u[hh * 64:(hh + 1) * 64, :],
                            lhsT=V_t[:, g0, h * 64:(h + 1) * 64],
                            rhs=e0[:, hh, :],
                            start=(g0 == 0), stop=(g0 == NJ - 1),
                            tile_position=(0, hh * 64),
                            skip_group_check=True,
                        )
                    if racc_split:
                        if g0 == 0:
                            nc.vector.tensor_copy(out=racc[:, 0, :], in_=e0[:, 0, :])
                            nc.gpsimd.tensor_copy(out=racc[:, 1, :], in_=e0[:, 1, :])
                        else:
                            nc.vector.tensor_add(
                                racc[:, 0, :], racc[:, 0, :], e0[:, 0, :]
                            )
                            nc.gpsimd.tensor_add(
                                racc[:, 1, :], racc[:, 1, :], e0[:, 1, :]
                            )
                    else:
                        if g0 == 0:
                            nc.vector.tensor_copy(out=racc[:], in_=e0[:])
                        else:
                            nc.vector.tensor_add(racc[:], racc[:], e0[:])
                    if g0 == NJ - 1:
                        rs_pair = []
                        at(G + 1, lambda racc=racc, rs=rs_pair: tail_rp(racc, rs))
                        at(G + 2,
                           lambda i0=i0, p0=p0, u=u, rs=rs_pair:
                           tail_fin(i0, p0, u, rs))
                for fn in sched.pop(G, ()):
                    fn()
                if G < NG:
                    s = s_tiles.pop(G)
                    e = epool.tile([P, 2, 512], f16, tag="e", name="e")
                    nc.scalar.activation(out=e[:], in_=s[:], func=EXP, scale=SCALE)
                    e_tiles[G] = e
                    if G + 2 < NG:
                        s_tiles[G + 2] = st_group(*order[G + 2])
            # final block's out-projection
            for m in range(4):
                for n2 in range(2):
                    out_proj(NI - 1, m, n2)

        if loop_iters is None:
            body()
        else:
            with tc.For_i(0, loop_iters, 1) as iv:
                body(iv)

    nc.compile()
    return nc


def _in_maps(x, w_qkv, w_out):
    in_maps = []
    x16 = np.asarray(x, dtype=np.float16)
    for bi in range(4):
        xTb = np.ascontiguousarray(x16[bi].T)
        for hg in range(2):
            c = slice(hg * 512, (hg + 1) * 512)
            wqk16 = np.ascontiguousarray(
                np.concatenate(
                    [w_qkv[:, c], w_qkv[:, 1024:2048][:, c]], axis=1
                ).astype(np.float16)
            )
            wv16 = np.ascontiguousarray(w_qkv[:, 2048:3072][:, c].astype(np.float16))
            wo16 = np.ascontiguousarray(w_out[c, :].astype(np.float16))
            in_maps.append({"xT": xTb, "wqk": wqk16, "wv": wv16, "wout": wo16})
    return in_maps


def kernel(x, w_qkv, w_out, b_out):
    from concourse.bass_utils import run_bass_kernel_spmd

    if "nc" not in _CACHE:
        _CACHE["nc"] = _build()
    nc = _CACHE["nc"]

    x = np.asarray(x, dtype=np.float32)
    w_qkv = np.asarray(w_qkv, dtype=np.float32)
    w_out = np.asarray(w_out, dtype=np.float32)
    b_out = np.asarray(b_out, dtype=np.float32)

    res = run_bass_kernel_spmd(
        nc, _in_maps(x, w_qkv, w_out), core_ids=list(range(8))
    )
    out = np.empty((4, NT, DIM), dtype=np.float32)
    for bi in range(4):
        out[bi] = (
            res.results[2 * bi]["y"].astype(np.float32)
            + res.results[2 * bi + 1]["y"].astype(np.float32)
            + b_out
        )
    return out


# revision 3
# speedup vs baseline: 1.5407x; 1.1277x over previous
"""Multi-head attention (dense_transformer) on 8 Trainium2 NeuronCores.

Reference computation (DIM=1024, HEADS=16, HEAD_DIM=64, SCALE=DIM**-0.5):
    qkv = x @ w_qkv ; q,k,v = split-heads ; attn = softmax(q k^T / 32)
    out = merge-heads(attn @ v) @ w_out + b_out

Sharding: 8 cores = (batch 0..3) x (head-group 0..1, 8 heads each); each core
emits a [2048, 1024] f16 partial of the output projection; host sums the two
head-group partials in fp32 and adds the bias.

v5 design notes:
  - f16 host-cast inputs, per-chunk loads ordered for a fast ramp, weights on
    the Activation HWDGE ring, x on the SP ring
  - flat software pipeline over (i-block, head-pair, j-chunk) groups:
    S^T pair (row-tiled PE) -> exp (ScalarE, N=1024) -> PV pair (col-tiled
    PE accumulate) with the S^T lookahead emitted FIRST each step so the
    s-ring WAR (S(G+2) <- exp(G)) hides behind the group's other PE work
  - softmax denominator: e-tiles are QUAD tiles (4 groups per tile); the
    running sum racc4 is updated once per quad (alternating DVE / GpSimd)
    instead of once per group -- 6 sync points per pair instead of 32.
    Pair tail folds racc4 on DVE, then ones-matmul reduce + reciprocal +
    K=1 broadcast + normalize, all deferred so they never head-block PE.
  - PSUM->SBUF copies (projections, out-proj) deferred one group so their
    semaphore waits are pre-satisfied when the DVE queue reaches them
"""

import numpy as np

P = 128
DIM = 1024
NT = 2048          # tokens per batch
HL = 8             # heads per core (local)
HD = 64
KD = DIM // P      # 8 contraction chunks for the projections
NI = NT // 512     # 4 query blocks of 512
NJ = NT // P       # 16 key chunks of 128
SCALE = DIM ** -0.5

_CACHE = {}


def _build(loop_iters=None, act_n=None, staggered=False, strip=None,
           racc_eng="mixed"):
    from contextlib import ExitStack

    import concourse.bacc as bacc
    import concourse.tile as tile
    from concourse import mybir

    f16 = mybir.dt.float16
    f32 = mybir.dt.float32
    EXP = mybir.ActivationFunctionType.Exp

    nc = bacc.Bacc("TRN2", target_bir_lowering=False, debug=False)

    xT = nc.dram_tensor("xT", [DIM, NT], f16, kind="ExternalInput").ap()
    wqk = nc.dram_tensor("wqk", [DIM, 1024], f16, kind="ExternalInput").ap()
    wv = nc.dram_tensor("wv", [DIM, 512], f16, kind="ExternalInput").ap()
    wout = nc.dram_tensor("wout", [512, 1024], f16, kind="ExternalInput").ap()
    y = nc.dram_tensor("y", [NT, 1024], f16, kind="ExternalOutput").ap()

    with tile.TileContext(nc) as tc, ExitStack() as ctx, nc.allow_low_precision(
        reason="f16 weights/activations + f16 partial outputs, validated vs reference"
    ):
        persist = ctx.enter_context(tc.tile_pool(name="persist", bufs=1))
        epool = ctx.enter_context(tc.tile_pool(name="epool", bufs=4))
        rpool = ctx.enter_context(tc.tile_pool(name="rpool", bufs=2))
        ypool = ctx.enter_context(tc.tile_pool(name="ypool", bufs=6))
        ps_s = ctx.enter_context(tc.tile_pool(name="ps_s", bufs=2, space="PSUM"))
        ps_u = ctx.enter_context(tc.tile_pool(name="ps_u", bufs=2, space="PSUM"))
        ps_r = ctx.enter_context(tc.tile_pool(name="ps_r", bufs=2, space="PSUM"))

        xT_t = persist.tile([P, KD, NT], f16)        # x^T
        qkT_t = persist.tile([P, KD, NT], f16)       # [Q^T; K^T]
        V_t = persist.tile([P, NJ, 512], f16)        # V natural layout
        U_t = persist.tile([P, 4, NT], f16)          # U^T normalized, pair-chunked
        wqk_t = persist.tile([P, KD, 1024], f16)
        wv_t = persist.tile([P, KD, 512], f16)
        wout_t = persist.tile([P, 4, 1024], f16)
        ones_r = persist.tile([P, 1], f16)           # K=128, M=1 column-sum
        ones_b = persist.tile([1, 64], f16)          # K=1 broadcast
        nc.vector.memset(ones_r, 1.0)
        nc.vector.memset(ones_b, 1.0)

        def body(_iv=None):
            # ---- loads ----
            weng = nc.scalar
            for k in range(KD):
                weng.dma_start(out=wqk_t[:, k, :], in_=wqk[k * P:(k + 1) * P, :])
            weng.dma_start(
                out=wv_t[:, :, :], in_=wv.rearrange("(k p) c -> p k c", p=P)
            )
            weng.dma_start(
                out=wout_t[:, :, :], in_=wout.rearrange("(k p) c -> p k c", p=P)
            )
            for k in range(KD):
                nc.sync.dma_start(
                    out=xT_t[:, k, 0:512], in_=xT[k * P:(k + 1) * P, 0:512]
                )
            for q in range(1, 4):
                for k in range(KD):
                    nc.sync.dma_start(
                        out=xT_t[:, k, q * 512:(q + 1) * 512],
                        in_=xT[k * P:(k + 1) * P, q * 512:(q + 1) * 512],
                    )

            def qk_chunk(m, n):
                ps = ps_r.tile([P, 512], f32, tag="rr", name="ps_qk")
                for k in range(KD):
                    nc.tensor.matmul(
                        ps,
                        lhsT=wqk_t[:, k, m * P:(m + 1) * P],
                        rhs=xT_t[:, k, n * 512:(n + 1) * 512],
                        start=(k == 0), stop=(k == KD - 1),
                    )
                return lambda: nc.vector.tensor_copy(
                    out=qkT_t[:, m, n * 512:(n + 1) * 512], in_=ps
                )

            def v_chunk(mt):
                ps = ps_r.tile([P, 512], f32, tag="rr", name="ps_v")
                for k in range(KD):
                    nc.tensor.matmul(
                        ps,
                        lhsT=xT_t[:, k, mt * P:(mt + 1) * P],
                        rhs=wv_t[:, k, :],
                        start=(k == 0), stop=(k == KD - 1),
                    )
                return lambda: nc.vector.tensor_copy(out=V_t[:, mt, :], in_=ps)

            order = [
                (i, p, g) for i in range(NI) for p in range(4) for g in range(NJ)
            ]
            NG = len(order)

            def st_group(i, p, g, s4, ph):
                isl = slice(i * 512, (i + 1) * 512)
                jsl = slice(g * P, (g + 1) * P)
                for hh in range(2):
                    pb = hh * 64
                    nc.tensor.matmul(
                        s4[:, hh, :],
                        lhsT=qkT_t[pb:pb + 64, 4 + p, jsl],
                        rhs=qkT_t[pb:pb + 64, p, isl],
                        start=True, stop=True,
                        tile_position=(pb, 0),
                    )

            sched = {}
            cur_g = [0]

            def at(G, fn):
                sched.setdefault(G, []).append(fn)

            def defer(fn, d=1):
                at(cur_g[0] + d, fn)

            def task(proj_fn, *args):
                def run():
                    defer(proj_fn(*args))
                return run

            tasks = []
            for p in range(4):
                for n in range(4):
                    if (p, n) != (0, 0):
                        tasks.append(
                            (max(0, n * 64 + p * 16 - 6), task(qk_chunk, p, n))
                        )
                        tasks.append(
                            (max(0, p * 16 + 4 * n - 6), task(qk_chunk, 4 + p, n))
                        )
            for mt in range(2, NJ):
                tasks.append((max(0, mt - 2), task(v_chunk, mt)))
            tasks.sort(key=lambda t: t[0])
            if strip in ("attn", "attn_min", "attn_notail"):
                tasks = []
            for dl, fn in tasks:
                at(dl, fn)

            def out_proj(i, m, n2):
                msl = slice(i * 512 + m * P, i * 512 + (m + 1) * P)
                py = ps_r.tile([P, 512], f32, tag="rr", name="py")
                for k in range(4):
                    nc.tensor.matmul(
                        py,
                        lhsT=U_t[:, k, msl],
                        rhs=wout_t[:, k, n2 * 512:(n2 + 1) * 512],
                        start=(k == 0), stop=(k == 3),
                    )

                def fin():
                    ysb = ypool.tile([P, 512], f16, tag="y", name="ysb")
                    nc.vector.tensor_copy(out=ysb, in_=py)
                    nc.sync.dma_start(
                        out=y[msl, n2 * 512:(n2 + 1) * 512], in_=ysb
                    )
                return fin

            if strip not in ("attn", "attn_min", "attn_notail"):
                for i in range(NI - 1):
                    for t, (m, n2) in enumerate(
                        [(m, n2) for m in range(4) for n2 in range(2)]
                    ):
                        at((i + 1) * 64 + 2 * t + 12, task(out_proj, i, m, n2))

            def tail_fold1(racc_d, racc_p):
                nc.vector.tensor_add(racc_d[:], racc_d[:], racc_p[:])

            def tail_fold2(racc_d):
                nc.vector.tensor_add(
                    racc_d[:, 0:2], racc_d[:, 0:2], racc_d[:, 2:4]
                )

            def tail_fold3(racc_d):
                nc.vector.tensor_add(
                    racc_d[:, 0], racc_d[:, 0], racc_d[:, 1]
                )

            def tail_rp(racc_d):
                rps = []
                for hh in range(2):
                    rp = ps_r.tile([P, 512], f32, tag="rr", name="rp")
                    nc.tensor.matmul(
                        rp[0:1, :], lhsT=ones_r, rhs=racc_d[:, 0, hh, :],
                        start=True, stop=True,
                    )
                    rps.append(rp)
                return rps

            def tail_recip(rps, rs_pair):
                for hh in range(2):
                    rs = rpool.tile([1, 512], f16, tag=f"rs{hh}", name="rs")
                    nc.vector.reciprocal(out=rs, in_=rps[hh][0:1, :])
                    rs_pair.append(rs)

            def tail_rb(rs_pair):
                rb = ps_r.tile([P, 512], f32, tag="rr", name="rb")
                for hh in range(2):
                    nc.tensor.matmul(
                        rb[hh * 64:(hh + 1) * 64, :],
                        lhsT=ones_b, rhs=rs_pair[hh],
                        start=True, stop=True,
                        tile_position=(0, hh * 64),
                    )
                return rb

            def tail_fin(i0, p0, u, rbbox):
                isl = slice(i0 * 512, (i0 + 1) * 512)
                rb_sb = rpool.tile([P, 512], f16, tag="rb", name="rb_sb")
                nc.vector.tensor_copy(out=rb_sb, in_=rbbox[0])
                nc.vector.tensor_mul(out=U_t[:, p0, isl], in0=u, in1=rb_sb)

            # ---- preamble ----
            if strip in ("attn", "attn_min", "attn_notail"):
                nc.vector.memset(qkT_t, 0.5)
                nc.vector.memset(V_t, 0.5)
            else:
                qk_chunk(0, 0)()
                qk_chunk(4, 0)()
                v_chunk(0)()
                v_chunk(1)()
            if strip == "proj":
                nc.vector.memset(U_t, 0.5)
                for G in range(NG + 16):
                    cur_g[0] = G
                    for fn in sched.pop(G, ()):
                        fn()
                for m in range(4):
                    for n2 in range(2):
                        out_proj(NI - 1, m, n2)()
                return

            # e/s quad tiles: one [P, 4, 2, 512] tile serves 4 consecutive
            # groups of a pair (phases 0..3)
            e4_tiles = {}   # quad index -> tile
            s_tiles = {}    # group -> s psum tile
            quads = {}      # quad-in-pair -> e4 tile
            racc_p = None
            racc_d = None

            def make_s(G):
                i, p, g = order[G]
                s = ps_s.tile([P, 2, 512], f32, tag="s", name="s_ps")
                s_tiles[G] = s
                st_group(i, p, g, s, g % 4)

            make_s(0)
            make_s(1)

            u = None
            # Pipeline: step G emits S^T(G+2) first (its WAR on exp(G) hides
            # behind this step's other PE work), then exp(G), then PV(G-1).
            for G in range(NG + 16):
                cur_g[0] = G
                if G + 2 < NG:
                    make_s(G + 2)
                if G < NG:
                    iG, pG, gG = order[G]
                    qi = G // 4
                    if gG % 4 == 0:
                        e4_tiles[qi] = epool.tile(
                            [P, 4, 2, 512], f16, tag="e", name="e4"
                        )
                    e4 = e4_tiles[qi]
                    s = s_tiles.pop(G)
                    if act_n is None:
                        nc.scalar.activation(
                            out=e4[:, gG % 4], in_=s[:], func=EXP, scale=SCALE
                        )
                    else:
                        nc.scalar.activation(
                            out=e4[:, gG % 4, 0, 0:act_n],
                            in_=s[:, 0, 0:act_n], func=EXP, scale=SCALE,
                        )
                if 1 <= G <= NG:
                    i0, p0, g0 = order[G - 1]
                    q0 = (G - 1) // 4
                    e0 = e4_tiles[q0]
                    ph = g0 % 4
                    if g0 == 0:
                        u = ps_u.tile([P, 512], f32, tag="u", name="u")
                    for hh in range(2):
                        h = 2 * p0 + hh
                        nc.tensor.matmul(
                            u[hh * 64:(hh + 1) * 64, :],
                            lhsT=V_t[:, g0, h * 64:(h + 1) * 64],
                            rhs=e0[:, ph, hh, :],
                            start=(g0 == 0), stop=(g0 == NJ - 1),
                            tile_position=(0, hh * 64),
                            skip_group_check=True,
                        )
                    if strip != "attn_min" and ph == 3:
                        # Quad complete. Two INDEPENDENT partial sums -- Pool
                        # owns quads 0+1, DVE owns quads 2+3 -- merged by a
                        # DVE fold tree at the pair tail. Each step of the
                        # tail chain is spaced >=1 group after its producer
                        # so its queue-head wait is pre-satisfied (blocking
                        # waits cost ~1us of wake-up latency on HW).
                        qq = g0 // 4        # quad index within pair (0..3)
                        quads[qq] = e0
                        if qq == 1:
                            # Pool sums quads 0+1 mid-pair, off the chain
                            racc_p = rpool.tile(
                                [P, 4, 2, 512], f16, tag="rp", name="racc_p"
                            )
                            peng = (nc.vector if racc_eng == "dve"
                                    else nc.gpsimd)
                            defer(lambda r=racc_p, a=quads[0], b=quads[1]:
                                  peng.tensor_add(r[:], a[:], b[:]), 2)
                        if qq == 3:
                            # DVE sums quads 2+3 right after pair end (fast)
                            racc_d = rpool.tile(
                                [P, 4, 2, 512], f16, tag="rd", name="racc_d"
                            )
                            defer(lambda r=racc_d, a=quads[2], b=quads[3]:
                                  nc.vector.tensor_add(r[:], a[:], b[:]), 2)
                        if qq == 3 and strip != "attn_notail":
                            rs_pair = []
                            rpbox = []
                            rbbox = []
                            at(G + 4, lambda d=racc_d, p=racc_p:
                               (tail_fold1(d, p), tail_fold2(d),
                                tail_fold3(d)))
                            at(G + 8, lambda d=racc_d, rb=rpbox:
                               rb.extend(tail_rp(d)))
                            at(G + 9, lambda rb=rpbox, rs=rs_pair:
                               tail_recip(rb, rs))
                            at(G + 10, lambda rs=rs_pair, bb=rbbox:
                               bb.append(tail_rb(rs)))
                            at(G + 11,
                               lambda i0=i0, p0=p0, u=u, bb=rbbox:
                               tail_fin(i0, p0, u, bb))
                for fn in sched.pop(G, ()):
                    fn()
            # final block's out-projection
            if strip is None:
                fins = []
                for m in range(4):
                    for n2 in range(2):
                        fins.append(out_proj(NI - 1, m, n2))
                        if len(fins) >= 2:
                            fins.pop(0)()
                for fn in fins:
                    fn()

        if loop_iters is None:
            body()
        else:
            with tc.For_i(0, loop_iters, 1, staggered_reset=staggered) as iv:
                body(iv)

    nc.compile()
    return nc


def _in_maps(x, w_qkv, w_out):
    in_maps = []
    x16 = np.asarray(x, dtype=np.float16)
    for bi in range(4):
        xTb = np.ascontiguousarray(x16[bi].T)
        for hg in range(2):
            c = slice(hg * 512, (hg + 1) * 512)
            wqk16 = np.ascontiguousarray(
                np.concatenate(
                    [w_qkv[:, c], w_qkv[:, 1024:2048][:, c]], axis=1
                ).astype(np.float16)
            )
            wv16 = np.ascontiguousarray(w_qkv[:, 2048:3072][:, c].astype(np.float16))
            wo16 = np.ascontiguousarray(w_out[c, :].astype(np.float16))
            in_maps.append({"xT": xTb, "wqk": wqk16, "wv": wv16, "wout": wo16})
    return in_maps


def kernel(x, w_qkv, w_out, b_out):
    from concourse.bass_utils import run_bass_kernel_spmd

    if "nc" not in _CACHE:
        _CACHE["nc"] = _build()
    nc = _CACHE["nc"]

    x = np.asarray(x, dtype=np.float32)
    w_qkv = np.asarray(w_qkv, dtype=np.float32)
    w_out = np.asarray(w_out, dtype=np.float32)
    b_out = np.asarray(b_out, dtype=np.float32)

    res = run_bass_kernel_spmd(
        nc, _in_maps(x, w_qkv, w_out), core_ids=list(range(8))
    )
    out = np.empty((4, NT, DIM), dtype=np.float32)
    for bi in range(4):
        out[bi] = (
            res.results[2 * bi]["y"].astype(np.float32)
            + res.results[2 * bi + 1]["y"].astype(np.float32)
            + b_out
        )
    return out
